# revision 3
# baseline (speedup 1.0000x reference)
"""MoE (top-K routing, per-expert capacity) Trainium2 kernel.

Strategy: expert parallelism across 8 NeuronCores (E=8, one expert per core).
 - Host: routing top-C selection per expert (tiny: E x T scores), gather of
   dispatched tokens, and fold of the combine weights ("gain") into the
   dispatched activations. gain >= 0 (softmax outputs), so
   gain * (relu(xe@W1)@W2) == relu((gain*xe)@W1)@W2 exactly in math terms.
 - Device (per core): fused 2-layer MLP in a single hand-written Tile kernel:
       hT = relu(W1.T @ xeT)   (F, Ca)   hT kept in SBUF, F in G groups
       y  = hT.T @ W2          (Ca, D)   PSUM-accumulated per group,
                                         DVE-accumulated across groups
 - Host: per-expert scatter-add of y_e back into the (T, D) output.

All tensors stream and compute in bf16 (PE streams 1 col/cycle for bf16 and
fp32r alike, but bf16 halves DMA bytes and LDWEIGHTS time, so weight loads
fully hide under the matmul stream). Pipeline error ~4e-3 vs the 2e-2 gate.
PSUM/combine accumulation stays fp32.

MM1 computes only the first c_cmp (= valid slots rounded to 8) of the c_act
columns; the trailing hT columns are zero garbage whose MM2 products land in
output rows the host discards (invalid-slot mask).

MM1 free-dim chunks are near-equal thirds (~380) instead of [512,384,256]:
every chunk's stream time (>=150ns) then covers the next 97ns bf16
LDWEIGHTS, so the weight port never stalls the PE.

DMA is split across the two TRN2 hardware DGE rings (Sync + Scalar
triggered): the startup xe stream rides both rings in single-ko slices, and
warm "filler" matmuls interleave fc0's k-steps so the PE never idles (and
the HAM clock gate never re-throttles) while xe lands.

b1/b2 are structurally zero in this problem (setup_inputs fills zeros); a
host-side fallback handles nonzero b2, and nonzero b1 is unsupported.
"""

import math
import sys

import numpy as np
import ml_dtypes

for _p in ("/opt/trn_rl_repo",):
    if _p not in sys.path:
        sys.path.append(_p)

BF16 = ml_dtypes.bfloat16

# Problem dims (hardcoded per contract)
T, E, D, F, C, K = 4096, 8, 1024, 4096, 1536, 2
N_CORES = 8
P = 128
G = 4  # F-dim groups for the fused hT staging
KO = D // P  # 8 k-subtiles of the D contraction
NF = F // P  # 32 f-chunks of 128
FPG = NF // G  # f-chunks per group
N_WARM = 9  # HAM warm-up matmuls while xe/w1 stream in
N_FILL = 2  # filler matmuls per fc0 k-step (bridge the xe stream-in)

_PROGRAMS = {}  # (c_act, c_cmp) -> (nc, names)


def _c_chunks(c_cmp):
    """Split c_cmp into near-equal matmul free-dim chunks <= 512 (PSUM bank
    limit), each a multiple of 8 columns (16B-aligned bf16 slices)."""
    n = (c_cmp + 511) // 512
    base = c_cmp // n
    chunks = []
    rem = c_cmp
    for i in range(n, 0, -1):
        take = min(512, ((rem + i - 1) // i + 7) // 8 * 8) if i > 1 else rem
        chunks.append(take)
        rem -= take
    assert sum(chunks) == c_cmp and all(0 < c <= 512 for c in chunks), chunks
    return chunks


def _build_program(c_act, c_cmp):
    import concourse.mybir as mybir
    import concourse.tile as tile
    from concourse import bacc

    f32 = mybir.dt.float32
    bf16 = mybir.dt.bfloat16
    Relu = mybir.ActivationFunctionType.Relu

    CS = c_act // P  # c-subtiles for MM2
    ND = D // 512  # 2 n-chunks of 512 for MM2
    chunks = _c_chunks(c_cmp)

    nc = bacc.Bacc(None, target_bir_lowering=False, debug=False)

    with tile.TileContext(nc) as tc:
        with tc.tile_pool(name="dram", bufs=1, space="DRAM") as dram:
            # w1 block-packed on host: (NF, P, KO, P); [fg] -> [ki, ko, f] tile
            w1 = dram.tile((NF, P, KO, P), bf16, kind="ExternalInput", name="w1")
            w2 = dram.tile((F, D), bf16, kind="ExternalInput", name="w2")
            xeT = dram.tile((D, c_act), bf16, kind="ExternalInput", name="xeT")
            y = dram.tile((c_act, D), bf16, kind="ExternalOutput", name="y")

        xeT_r = xeT[:].rearrange("(ko ki) c -> ki ko c", ki=P)
        w2_r = w2[:].rearrange("(nf p) d -> p nf d", p=P)

        with (
            tc.tile_pool(name="const", bufs=1) as constp,
            tc.tile_pool(name="xe", bufs=1) as xep,
            tc.tile_pool(name="ht", bufs=1) as htp,
            tc.tile_pool(name="ysb", bufs=1) as yp,
            tc.tile_pool(name="w1t", bufs=10) as w1p,
            tc.tile_pool(name="w2t", bufs=2) as w2p,
            tc.tile_pool(name="ps", bufs=2, space="PSUM") as psp,
            tc.tile_pool(name="warmps", bufs=1, space="PSUM") as warmp,
        ):
            # HAM warm-up operands: memset off the Scalar queue (its
            # ACT_TABLE_LOAD would delay the first warm matmul)
            warm_w = constp.tile([P, P], bf16)
            nc.gpsimd.memset(warm_w[:], 0.0)
            warm_sb = constp.tile([P, 512], bf16)
            nc.vector.memset(warm_sb[:], 0.0)
            zero = constp.tile([P, 1], f32)
            nc.gpsimd.memset(zero[:], 0.0)
            warm_out = constp.tile([P, 1], f32)

            hT = htp.tile([P, FPG, c_act], bf16)
            if c_cmp < c_act:
                # tail hT columns are never computed; zero them so MM2's
                # last c-subtile multiplies defined data (its output rows
                # are invalid slots the host discards anyway)
                nc.gpsimd.memset(hT[:, :, c_cmp:], 0.0)

            # Startup-critical loads split across both DGE rings: first
            # stationary W1 tile + xe ko 0-3 on sync, xe ko 4-7 on scalar.
            # Single-ko xe slices land progressively so fc0's k-loop paces
            # with the stream.
            w1_first = w1p.tile([P, KO, P], bf16, name="w1_t")
            nc.sync.dma_start(w1_first[:], w1[0])
            xe_sb = xep.tile([P, KO, c_act], bf16)
            for ko in range(KO):
                eng = nc.sync if ko < (KO + 1) // 2 else nc.scalar
                eng.dma_start(xe_sb[:, ko, :], xeT_r[:, ko, :])

            # HAM warm-up: dependency-free matmuls fill the PE activity
            # window during the xe/w1 stream-in, so real matmuls start at
            # 2.4GHz instead of the cold 1.2GHz
            warm_ps = warmp.tile([P, 512], f32)
            for _ in range(N_WARM):
                nc.tensor.matmul(
                    warm_ps[:], warm_w[:], warm_sb[:], start=True, stop=True
                )

            y_sb = yp.tile([P, CS, D], f32)
            y_bf = yp.tile([P, CS, D], bf16)

            # chunk index -> (c offset, width)
            offs = []
            c0 = 0
            for cw in chunks:
                offs.append((c0, cw))
                c0 += cw
            idxs = list(range(len(chunks)))

            def mm1_sweep(g, use_first):
                """One fc-sweep of MM1 over all c-chunks."""
                for fc in range(FPG):
                    fg = g * FPG + fc
                    if use_first and fc == 0:
                        w1_t = w1_first
                    else:
                        w1_t = w1p.tile([P, KO, P], bf16, name="w1_t")
                        # fg 1-2 ride the sync ring (needed before the
                        # scalar ring works through its xe half); the rest
                        # stream on the scalar ring
                        eng = nc.sync if fg <= 2 else nc.scalar
                        eng.dma_start(w1_t[:], w1[fg])
                    ph = {
                        i: psp.tile([P, chunks[i]], f32, name=f"p{i}", tag=f"p{i}")
                        for i in idxs
                    }
                    for k in range(KO):
                        for i in idxs:
                            c0, cw = offs[i]
                            nc.tensor.matmul(
                                ph[i][:],
                                w1_t[:, k, :],
                                xe_sb[:, k, c0 : c0 + cw],
                                start=(k == 0),
                                stop=(k == KO - 1),
                            )
                        if use_first and fc == 0 and k < KO - 1:
                            # keep the PE busy (and the HAM gate open)
                            # while the next xe slice lands
                            for _ in range(N_FILL):
                                nc.tensor.matmul(
                                    warm_ps[:],
                                    warm_w[:],
                                    warm_sb[:],
                                    start=True,
                                    stop=True,
                                )
                    for i in idxs:
                        c0, cw = offs[i]
                        nc.scalar.activation(
                            hT[:, fc, c0 : c0 + cw], ph[i][:], Relu, bias=zero[:]
                        )

            def mm2_cs(cs, w2_t, g, dh_list):
                """MM2 accumulation over this group's f-chunks for one
                c-subtile, then fold into y_sb / store via y_bf."""
                py = {
                    dh: psp.tile([P, 512], f32, name=f"py{dh}", tag=f"p{dh}")
                    for dh in dh_list
                }
                for fs in range(FPG):
                    for dh in dh_list:
                        nc.tensor.matmul(
                            py[dh][:],
                            hT[:, fs, cs * P : (cs + 1) * P],
                            w2_t[:, fs, dh * 512 : (dh + 1) * 512],
                            start=(fs == 0),
                            stop=(fs == FPG - 1),
                        )
                for dh in dh_list:
                    sl = slice(dh * 512, (dh + 1) * 512)
                    if g == 0:
                        nc.vector.tensor_copy(y_sb[:, cs, sl], py[dh][:])
                    elif g < G - 1:
                        nc.vector.tensor_add(
                            y_sb[:, cs, sl], y_sb[:, cs, sl], py[dh][:]
                        )
                    else:
                        # final group: fold the add into a bf16 store tile
                        nc.vector.tensor_add(
                            y_bf[:, cs, sl], y_sb[:, cs, sl], py[dh][:]
                        )

            for g in range(G):
                # ---- MM1: hT[group] = relu(W1[:, group].T @ xeT) ----
                mm1_sweep(g, use_first=(g == 0))

                # W2 for this group: one batched transfer on the sync ring
                # (emitted after MM1 so the ring serves w1/xe first)
                w2_t = w2p.tile([P, FPG, D], bf16, name="w2_t")
                nc.sync.dma_start(w2_t[:], w2_r[:, g * FPG : (g + 1) * FPG, :])

                # ---- MM2: y[group contribution] = hT.T @ W2[group] ----
                for cs in range(CS):
                    last = g == G - 1
                    if last and cs == CS - 1:
                        # final c-subtile: run the two D-halves as separate
                        # accumulation passes so the first half's add+store
                        # hides under the second half's matmul stream
                        for dh in range(ND):
                            mm2_cs(cs, w2_t, g, [dh])
                            half = slice(dh * 512, (dh + 1) * 512)
                            eng = nc.sync if dh % 2 == 0 else nc.scalar
                            eng.dma_start(
                                y[cs * P : (cs + 1) * P, half],
                                y_bf[:, cs, half],
                            )
                    else:
                        mm2_cs(cs, w2_t, g, list(range(ND)))
                        if last:
                            # one batched store per c-subtile, alternating
                            # rings so triggers don't serialize
                            eng = nc.sync if cs % 2 == 0 else nc.scalar
                            eng.dma_start(
                                y[cs * P : (cs + 1) * P, :], y_bf[:, cs, :]
                            )

    nc.compile()
    names = dict(w1=w1.name, w2=w2.name, xeT=xeT.name, y=y.name)
    return nc, names


def _get_program(c_act, c_cmp):
    key = (c_act, c_cmp)
    if key not in _PROGRAMS:
        _PROGRAMS[key] = _build_program(c_act, c_cmp)
    return _PROGRAMS[key]


# test.py can set RUN_KWARGS (e.g. dict(trace=True)) and read LAST_RESULTS
RUN_KWARGS = {}
LAST_RESULTS = None


def kernel(x, route_mask, route_weight, W1, b1, W2, b2):
    from concourse.bass_utils import run_bass_kernel_spmd

    global LAST_RESULTS

    x = np.asarray(x, dtype=np.float32)
    route_mask = np.asarray(route_mask, dtype=bool)
    route_weight = np.asarray(route_weight, dtype=np.float32)
    W1 = np.asarray(W1, dtype=np.float32)
    W2 = np.asarray(W2, dtype=np.float32)
    b1 = np.asarray(b1, dtype=np.float32)
    b2 = np.asarray(b2, dtype=np.float32)
    if np.any(b1):
        raise NotImplementedError("nonzero b1 not supported")

    # --- routing: per-expert top-C tokens by route weight (ties -> lower idx) ---
    w_et = np.where(route_mask.T, route_weight.T, -np.inf)  # (E, T)
    order = np.argsort(-w_et, axis=1, kind="stable")[:, :C]  # (E, C) token ids
    vals = np.take_along_axis(w_et, order, axis=1)  # (E, C)
    valid = np.isfinite(vals)  # (E, C)
    gain = np.where(valid, vals, 0.0).astype(np.float32)  # (E, C)

    # active capacity: valid slots are a prefix (sorted by weight desc).
    # c_act (tile shapes) is 128-aligned; c_cmp (MM1 computed columns) is
    # 8-aligned
    n_e = valid.sum(axis=1)
    n_max = int(max(1, n_e.max()))
    c_act = min(C, (n_max + P - 1) // P * P)
    c_cmp = min(c_act, (n_max + 7) // 8 * 8)

    nc, names = _get_program(c_act, c_cmp)

    # --- dispatch: gather + fold gain, per expert ---
    in_maps = []
    for e in range(E):
        xe = x[order[e, :c_act]] * gain[e, :c_act][:, None]  # (Ca, D)
        xeT_np = np.ascontiguousarray(xe.T).astype(BF16)  # (D, Ca)
        w1b = np.ascontiguousarray(
            W1[e].reshape(KO, P, NF, P).transpose(2, 1, 0, 3)
        ).astype(BF16)  # (NF, P, KO, P)
        in_maps.append(
            {
                names["w1"]: w1b,
                names["xeT"]: xeT_np,
                names["w2"]: W2[e].astype(BF16),
            }
        )

    res = run_bass_kernel_spmd(nc, in_maps, list(range(N_CORES)), **RUN_KWARGS)
    LAST_RESULTS = res

    # --- combine: scatter-add per-expert outputs ---
    y = np.zeros((T, D), np.float32)
    for e in range(E):
        ye = np.asarray(res.results[e][names["y"]]).astype(np.float32)  # (Ca, D)
        m = valid[e, :c_act]
        if np.any(b2):
            ye = ye + gain[e, :c_act][:, None] * b2[e][None, :]
        y[order[e, :c_act][m]] += ye[m]
    return y


# revision 9
# speedup vs baseline: 1.0200x; 1.0200x over previous
"""MoE (top-K routing, per-expert capacity) Trainium2 kernel.

Strategy: expert parallelism across 8 NeuronCores (E=8, one expert per core).
 - Host: routing top-C selection per expert (tiny: E x T scores), gather of
   dispatched tokens, and fold of the combine weights ("gain") into the
   dispatched activations. gain >= 0 (softmax outputs), so
   gain * (relu(xe@W1)@W2) == relu((gain*xe)@W1)@W2 exactly in math terms.
 - Device (per core): fused 2-layer MLP in a single hand-written Tile kernel:
       hT = relu(W1.T @ xeT)   (F, Ca)   hT kept in SBUF, F in G groups
       y  = hT.T @ W2          (Ca, D)   PSUM-accumulated per group,
                                         DVE-accumulated across groups
 - Host: per-expert scatter-add of y_e back into the (T, D) output.

All tensors stream and compute in bf16 (PE streams 1 col/cycle for bf16 and
fp32r alike, but bf16 halves DMA bytes and LDWEIGHTS time, so weight loads
fully hide under the matmul stream). Pipeline error ~4e-3 vs the 2e-2 gate.
PSUM/combine accumulation stays fp32.

MM1 computes only the first c_cmp (= valid slots rounded to 8) of the c_act
columns; the trailing hT columns are zeroed so MM2's products there land in
output rows the host discards (invalid-slot mask).

MM1 free-dim chunks are near-equal thirds (~380) instead of [512,384,256]:
every chunk's stream time (>=150ns) then covers the next 97ns bf16
LDWEIGHTS, so the weight port never stalls the PE.

MM1 processes f-tiles in PAIRS with a k-major inner loop: each arriving xe
k-slice feeds ~950ns of matmuls (2 f-tiles x 3 chunks), which paces the PE
exactly with the startup xe DMA stream - no idle gaps, so the HAM clock
gate stays open from the warm-up onward.

DMA triggers are split across the two TRN2 hardware DGE rings (Sync +
Scalar-engine triggered) and batched (w1 in fg-pairs, w2 per group, y per
cs-pair): each dynamic-DMA trigger costs ~620ns of engine-queue time and
one semaphore, and the kernel-exit epilogue zeroes every allocated
semaphore one by one.

b1/b2 are structurally zero in this problem (setup_inputs fills zeros); a
host-side fallback handles nonzero b2, and nonzero b1 is unsupported.
"""

import math
import sys

import numpy as np
import ml_dtypes

for _p in ("/opt/trn_rl_repo",):
    if _p not in sys.path:
        sys.path.append(_p)

BF16 = ml_dtypes.bfloat16

# Problem dims (hardcoded per contract)
T, E, D, F, C, K = 4096, 8, 1024, 4096, 1536, 2
N_CORES = 8
P = 128
G = 4  # F-dim groups for the fused hT staging
KO = D // P  # 8 k-subtiles of the D contraction
NF = F // P  # 32 f-chunks of 128
FPG = NF // G  # f-chunks per group
N_WARM = 9  # HAM warm-up matmuls while xe/w1 stream in

_PROGRAMS = {}  # (c_act, c_cmp) -> (nc, names)


def _c_chunks(c_cmp):
    """Split c_cmp into near-equal matmul free-dim chunks <= 512 (PSUM bank
    limit), each a multiple of 8 columns (16B-aligned bf16 slices)."""
    n = (c_cmp + 511) // 512
    chunks = []
    rem = c_cmp
    for i in range(n, 0, -1):
        take = min(512, ((rem + i - 1) // i + 7) // 8 * 8) if i > 1 else rem
        chunks.append(take)
        rem -= take
    assert sum(chunks) == c_cmp and all(0 < c <= 512 for c in chunks), chunks
    return chunks


def _build_program(c_act, c_cmp):
    import concourse.mybir as mybir
    import concourse.tile as tile
    from concourse import bacc

    f32 = mybir.dt.float32
    bf16 = mybir.dt.bfloat16
    Relu = mybir.ActivationFunctionType.Relu

    CS = c_act // P  # c-subtiles for MM2
    ND = D // 512  # 2 n-chunks of 512 for MM2
    chunks = _c_chunks(c_cmp)
    NPR = (FPG + 1) // 2  # f-tile pairs per group

    nc = bacc.Bacc(None, target_bir_lowering=False, debug=False)

    with tile.TileContext(nc) as tc:
        with tc.tile_pool(name="dram", bufs=1, space="DRAM") as dram:
            # w1 block-packed on host: (NF, P, KO, P); [fg] -> [ki, ko, f] tile
            w1 = dram.tile((NF, P, KO, P), bf16, kind="ExternalInput", name="w1")
            w2 = dram.tile((F, D), bf16, kind="ExternalInput", name="w2")
            xeT = dram.tile((D, c_act), bf16, kind="ExternalInput", name="xeT")
            y = dram.tile((c_act, D), bf16, kind="ExternalOutput", name="y")

        xeT_r = xeT[:].rearrange("(ko ki) c -> ki ko c", ki=P)
        w1_r = w1[:].rearrange("(np two) ki ko f -> np ki two ko f", two=2)
        w2_r = w2[:].rearrange("(nf p) d -> p nf d", p=P)

        with (
            tc.tile_pool(name="const", bufs=1) as constp,
            tc.tile_pool(name="xe", bufs=1) as xep,
            tc.tile_pool(name="ht", bufs=1) as htp,
            tc.tile_pool(name="ysb", bufs=1) as yp,
            tc.tile_pool(name="w1t", bufs=2) as w1p,
            tc.tile_pool(name="w2t", bufs=2) as w2p,
            tc.tile_pool(name="ps", bufs=2, space="PSUM") as psp,
            tc.tile_pool(name="warmps", bufs=1, space="PSUM") as warmp,
        ):
            # HAM warm-up operands: memset off the Scalar queue (its
            # ACT_TABLE_LOAD would delay the first warm matmul)
            warm_w = constp.tile([P, P], bf16)
            nc.gpsimd.memset(warm_w[:], 0.0)
            warm_sb = constp.tile([P, 512], bf16)
            nc.vector.memset(warm_sb[:], 0.0)
            zero = constp.tile([P, 1], f32)
            nc.gpsimd.memset(zero[:], 0.0)

            hT = htp.tile([P, FPG, c_act], bf16)
            if c_cmp < c_act:
                # tail hT columns are never computed; zero them so MM2's
                # last c-subtile multiplies defined data (its output rows
                # are invalid slots the host discards anyway)
                nc.gpsimd.memset(hT[:, :, c_cmp:], 0.0)

            # Startup-critical loads: the first W1 fg-pair is the scalar
            # ring's only early transfer; the SYNC ring is dedicated to the
            # xe stream (ko-pair slices, consumed ascending - each slice
            # feeds one pair-k-step burst).
            w1_first = w1p.tile([P, 2, KO, P], bf16, name="w1_t")
            nc.scalar.dma_start(w1_first[:], w1_r[0])
            xe_sb = xep.tile([P, KO, c_act], bf16)
            ko_step = 2 if KO % 2 == 0 else 1
            for kp in range(0, KO, ko_step):
                nc.sync.dma_start(
                    xe_sb[:, kp : kp + ko_step, :], xeT_r[:, kp : kp + ko_step, :]
                )

            # HAM warm-up: dependency-free matmuls fill the PE activity
            # window during the xe/w1 stream-in, so real matmuls start at
            # 2.4GHz instead of the cold 1.2GHz
            warm_ps = warmp.tile([P, 512], f32)
            for _ in range(N_WARM):
                nc.tensor.matmul(
                    warm_ps[:], warm_w[:], warm_sb[:], start=True, stop=True
                )

            y_sb = yp.tile([P, CS, D], f32)
            y_bf = yp.tile([P, CS, D], bf16)

            # chunk index -> (c offset, width)
            offs = []
            c0 = 0
            for cw in chunks:
                offs.append((c0, cw))
                c0 += cw
            idxs = list(range(len(chunks)))

            def mm1_sweep(g, use_first):
                """MM1 over this group's f-tiles, processed in pairs with a
                k-major inner loop (keeps the PE paced with the xe stream
                during startup; steady-state cycle count is unchanged)."""
                for pr in range(NPR):
                    fcs = [fc for fc in (2 * pr, 2 * pr + 1) if fc < FPG]
                    npr_g = g * NPR + pr
                    if use_first and pr == 0:
                        w1_t = w1_first
                    else:
                        w1_t = w1p.tile([P, 2, KO, P], bf16, name="w1_t")
                        # the 2nd pair rides the sync ring behind the xe
                        # slices (lands just before it's needed, without
                        # stealing early bandwidth); the rest stream on the
                        # scalar ring paced by pool recycling
                        eng = nc.sync if npr_g == 1 else nc.scalar
                        eng.dma_start(w1_t[:], w1_r[npr_g])
                    ph = {
                        (fc, i): psp.tile(
                            [P, chunks[i]], f32, name=f"p{i}", tag=f"p{i}"
                        )
                        for fc in fcs
                        for i in idxs
                    }
                    for k in range(KO):
                        for fc in fcs:
                            for i in idxs:
                                c0, cw = offs[i]
                                nc.tensor.matmul(
                                    ph[(fc, i)][:],
                                    w1_t[:, fc % 2, k, :],
                                    xe_sb[:, k, c0 : c0 + cw],
                                    start=(k == 0),
                                    stop=(k == KO - 1),
                                )
                    for fc in fcs:
                        for i in idxs:
                            c0, cw = offs[i]
                            nc.scalar.activation(
                                hT[:, fc, c0 : c0 + cw],
                                ph[(fc, i)][:],
                                Relu,
                                bias=zero[:],
                            )

            def mm2_cs(cs, w2_t, g, dh_list):
                """MM2 accumulation over this group's f-chunks for one
                c-subtile, then fold into y_sb (bf16 y_bf on last group)."""
                py = {
                    dh: psp.tile([P, 512], f32, name=f"py{dh}", tag=f"p{dh}")
                    for dh in dh_list
                }
                for fs in range(FPG):
                    for dh in dh_list:
                        nc.tensor.matmul(
                            py[dh][:],
                            hT[:, fs, cs * P : (cs + 1) * P],
                            w2_t[:, fs, dh * 512 : (dh + 1) * 512],
                            start=(fs == 0),
                            stop=(fs == FPG - 1),
                        )
                for dh in dh_list:
                    sl = slice(dh * 512, (dh + 1) * 512)
                    if g == 0:
                        nc.vector.tensor_copy(y_sb[:, cs, sl], py[dh][:])
                    elif g < G - 1:
                        nc.vector.tensor_add(
                            y_sb[:, cs, sl], y_sb[:, cs, sl], py[dh][:]
                        )
                    else:
                        nc.vector.tensor_add(
                            y_bf[:, cs, sl], y_sb[:, cs, sl], py[dh][:]
                        )

            for g in range(G):
                # ---- MM1: hT[group] = relu(W1[:, group].T @ xeT) ----
                mm1_sweep(g, use_first=(g == 0))

                # W2 for this group: one batched transfer on the sync ring
                # (emitted after MM1 so the ring serves w1/xe first)
                w2_t = w2p.tile([P, FPG, D], bf16, name="w2_t")
                nc.sync.dma_start(w2_t[:], w2_r[:, g * FPG : (g + 1) * FPG, :])

                # ---- MM2: y[group contribution] = hT.T @ W2[group] ----
                for cs in range(CS):
                    last = g == G - 1
                    if last and cs == CS - 1:
                        # final c-subtile: run the two D-halves as separate
                        # accumulation passes so the first half's add+store
                        # hides under the second half's matmul stream
                        for dh in range(ND):
                            mm2_cs(cs, w2_t, g, [dh])
                            half = slice(dh * 512, (dh + 1) * 512)
                            eng = nc.sync if dh % 2 == 0 else nc.scalar
                            eng.dma_start(
                                y[cs * P : (cs + 1) * P, half],
                                y_bf[:, cs, half],
                            )
                    else:
                        mm2_cs(cs, w2_t, g, list(range(ND)))
                        if last:
                            # batched store per c-subtile pair (alternating
                            # rings); a leftover even subtile before the
                            # final one stores alone
                            if cs % 2 == 1:
                                eng = nc.sync if cs % 4 == 1 else nc.scalar
                                # dst rearranged so element order matches the
                                # SBUF [p][two][d] tile layout
                                eng.dma_start(
                                    y[(cs - 1) * P : (cs + 1) * P, :].rearrange(
                                        "(two p) d -> p two d", p=P
                                    ),
                                    y_bf[:, cs - 1 : cs + 1, :],
                                )
                            elif cs == CS - 2:
                                nc.scalar.dma_start(
                                    y[cs * P : (cs + 1) * P, :],
                                    y_bf[:, cs, :],
                                )

    nc.compile()
    names = dict(w1=w1.name, w2=w2.name, xeT=xeT.name, y=y.name)
    return nc, names


def _get_program(c_act, c_cmp):
    key = (c_act, c_cmp)
    if key not in _PROGRAMS:
        _PROGRAMS[key] = _build_program(c_act, c_cmp)
    return _PROGRAMS[key]


# test.py can set RUN_KWARGS (e.g. dict(trace=True)) and read LAST_RESULTS
RUN_KWARGS = {}
LAST_RESULTS = None


def kernel(x, route_mask, route_weight, W1, b1, W2, b2):
    from concourse.bass_utils import run_bass_kernel_spmd

    global LAST_RESULTS

    x = np.asarray(x, dtype=np.float32)
    route_mask = np.asarray(route_mask, dtype=bool)
    route_weight = np.asarray(route_weight, dtype=np.float32)
    W1 = np.asarray(W1, dtype=np.float32)
    W2 = np.asarray(W2, dtype=np.float32)
    b1 = np.asarray(b1, dtype=np.float32)
    b2 = np.asarray(b2, dtype=np.float32)
    if np.any(b1):
        raise NotImplementedError("nonzero b1 not supported")

    # --- routing: per-expert top-C tokens by route weight (ties -> lower idx) ---
    w_et = np.where(route_mask.T, route_weight.T, -np.inf)  # (E, T)
    order = np.argsort(-w_et, axis=1, kind="stable")[:, :C]  # (E, C) token ids
    vals = np.take_along_axis(w_et, order, axis=1)  # (E, C)
    valid = np.isfinite(vals)  # (E, C)
    gain = np.where(valid, vals, 0.0).astype(np.float32)  # (E, C)

    # active capacity: valid slots are a prefix (sorted by weight desc).
    # c_act (tile shapes) is 128-aligned; c_cmp (MM1 computed columns) is
    # 8-aligned. The CS=1 case keeps a whole c-subtile so MM2 shapes hold.
    n_e = valid.sum(axis=1)
    n_max = int(max(1, n_e.max()))
    c_act = min(C, (n_max + P - 1) // P * P)
    c_cmp = min(c_act, (n_max + 7) // 8 * 8)

    nc, names = _get_program(c_act, c_cmp)

    # --- dispatch: gather + fold gain, per expert ---
    in_maps = []
    for e in range(E):
        xe = x[order[e, :c_act]] * gain[e, :c_act][:, None]  # (Ca, D)
        xeT_np = np.ascontiguousarray(xe.T).astype(BF16)  # (D, Ca)
        w1b = np.ascontiguousarray(
            W1[e].reshape(KO, P, NF, P).transpose(2, 1, 0, 3)
        ).astype(BF16)  # (NF, P, KO, P)
        in_maps.append(
            {
                names["w1"]: w1b,
                names["xeT"]: xeT_np,
                names["w2"]: W2[e].astype(BF16),
            }
        )

    res = run_bass_kernel_spmd(nc, in_maps, list(range(N_CORES)), **RUN_KWARGS)
    LAST_RESULTS = res

    # --- combine: scatter-add per-expert outputs ---
    y = np.zeros((T, D), np.float32)
    for e in range(E):
        ye = np.asarray(res.results[e][names["y"]]).astype(np.float32)  # (Ca, D)
        m = valid[e, :c_act]
        if np.any(b2):
            ye = ye + gain[e, :c_act][:, None] * b2[e][None, :]
        y[order[e, :c_act][m]] += ye[m]
    return y


# revision 14
# speedup vs baseline: 1.0327x; 1.0124x over previous
"""MoE (top-K routing, per-expert capacity) Trainium2 kernel.

Strategy: expert parallelism across 8 NeuronCores (E=8, one expert per core).
 - Host: routing top-C selection per expert (tiny: E x T scores), gather of
   dispatched tokens, and fold of the combine weights ("gain") into the
   dispatched activations. gain >= 0 (softmax outputs), so
   gain * (relu(xe@W1)@W2) == relu((gain*xe)@W1)@W2 exactly in math terms.
 - Device (per core): fused 2-layer MLP in a single hand-written Tile kernel:
       hT = relu(W1.T @ xeT)   (F, Ca)   hT kept in SBUF, F in G groups
       y  = hT.T @ W2          (Ca, D)   PSUM-accumulated per group,
                                         DVE-accumulated across groups
 - Host: per-expert scatter-add of y_e back into the (T, D) output.

All tensors stream and compute in bf16 (PE streams 1 col/cycle for bf16 and
fp32r alike, but bf16 halves DMA bytes and LDWEIGHTS time, so weight loads
fully hide under the matmul stream). Pipeline error ~4e-3 vs the 2e-2 gate.
PSUM/combine accumulation stays fp32.

MM1 computes only the first c_cmp (= valid slots rounded to 8) of the c_act
columns; the trailing hT columns are zeroed so MM2's products there land in
output rows the host discards (invalid-slot mask).

MM1 free-dim chunks are near-equal thirds (~380) instead of [512,384,256]:
every chunk's stream time (>=150ns) then covers the next 97ns bf16
LDWEIGHTS, so the weight port never stalls the PE.

MM1 processes f-tiles in PAIRS with a k-major inner loop: each arriving xe
k-slice feeds ~950ns of matmuls (2 f-tiles x 3 chunks), which paces the PE
exactly with the startup xe DMA stream - no idle gaps, so the HAM clock
gate stays open from the warm-up onward.

DMA triggers are split across the two TRN2 hardware DGE rings (Sync +
Scalar-engine triggered) and batched (w1 in fg-pairs, w2 per group, y per
cs-pair): each dynamic-DMA trigger costs ~620ns of engine-queue time and
one semaphore, and the kernel-exit epilogue zeroes every allocated
semaphore one by one.

b1/b2 are structurally zero in this problem (setup_inputs fills zeros); a
host-side fallback handles nonzero b2, and nonzero b1 is unsupported.
"""

import math
import sys

import numpy as np
import ml_dtypes

for _p in ("/opt/trn_rl_repo",):
    if _p not in sys.path:
        sys.path.append(_p)

BF16 = ml_dtypes.bfloat16

# Problem dims (hardcoded per contract)
T, E, D, F, C, K = 4096, 8, 1024, 4096, 1536, 2
N_CORES = 8
P = 128
G = 4  # F-dim groups for the fused hT staging
KO = D // P  # 8 k-subtiles of the D contraction
NF = F // P  # 32 f-chunks of 128
FPG = NF // G  # f-chunks per group
N_WARM = 12  # HAM warm-up matmuls while xe/w1 stream in

_PROGRAMS = {}  # (c_act, c_cmp) -> (nc, names)


def _c_chunks(c_cmp):
    """Split c_cmp into near-equal matmul free-dim chunks <= 512 (PSUM bank
    limit), each a multiple of 8 columns (16B-aligned bf16 slices)."""
    n = (c_cmp + 511) // 512
    chunks = []
    rem = c_cmp
    for i in range(n, 0, -1):
        take = min(512, ((rem + i - 1) // i + 7) // 8 * 8) if i > 1 else rem
        chunks.append(take)
        rem -= take
    assert sum(chunks) == c_cmp and all(0 < c <= 512 for c in chunks), chunks
    return chunks


def _build_program(c_act, c_cmp):
    import concourse.mybir as mybir
    import concourse.tile as tile
    from concourse import bacc

    f32 = mybir.dt.float32
    bf16 = mybir.dt.bfloat16
    Relu = mybir.ActivationFunctionType.Relu

    CS = c_act // P  # c-subtiles for MM2
    ND = D // 512  # 2 n-chunks of 512 for MM2
    chunks = _c_chunks(c_cmp)
    NPR = (FPG + 1) // 2  # f-tile pairs per group

    nc = bacc.Bacc(None, target_bir_lowering=False, debug=False)

    with tile.TileContext(nc) as tc:
        with tc.tile_pool(name="dram", bufs=1, space="DRAM") as dram:
            # w1 block-packed on host: (NF, P, KO, P); [fg] -> [ki, ko, f] tile
            w1 = dram.tile((NF, P, KO, P), bf16, kind="ExternalInput", name="w1")
            w2 = dram.tile((F, D), bf16, kind="ExternalInput", name="w2")
            xeT = dram.tile((D, c_act), bf16, kind="ExternalInput", name="xeT")
            y = dram.tile((c_act, D), bf16, kind="ExternalOutput", name="y")

        xeT_r = xeT[:].rearrange("(ko ki) c -> ki ko c", ki=P)
        w1_r = w1[:].rearrange("(np two) ki ko f -> np ki two ko f", two=2)
        w2_r = w2[:].rearrange("(nf p) d -> p nf d", p=P)

        with (
            tc.tile_pool(name="const", bufs=1) as constp,
            tc.tile_pool(name="xe", bufs=1) as xep,
            tc.tile_pool(name="ht", bufs=1) as htp,
            tc.tile_pool(name="ysb", bufs=1) as yp,
            tc.tile_pool(name="w1t", bufs=3) as w1p,
            tc.tile_pool(name="w2t", bufs=2) as w2p,
            tc.tile_pool(name="ps", bufs=2, space="PSUM") as psp,
            tc.tile_pool(name="warmps", bufs=1, space="PSUM") as warmp,
        ):
            # HAM warm-up operands: memset off the Scalar queue (its
            # ACT_TABLE_LOAD would delay the first warm matmul)
            warm_w = constp.tile([P, P], bf16)
            nc.gpsimd.memset(warm_w[:], 0.0)
            warm_sb = constp.tile([P, 512], bf16)
            nc.vector.memset(warm_sb[:], 0.0)
            zero = constp.tile([P, 1], f32)
            nc.gpsimd.memset(zero[:], 0.0)

            hT = htp.tile([P, FPG, c_act], bf16)
            if c_cmp < c_act:
                # tail hT columns are never computed; zero them so MM2's
                # last c-subtile multiplies defined data (its output rows
                # are invalid slots the host discards anyway)
                nc.gpsimd.memset(hT[:, :, c_cmp:], 0.0)

            # Startup-critical loads: the first W1 fg-pair is the scalar
            # ring's only early transfer; the SYNC ring is dedicated to the
            # xe stream (ko-pair slices, consumed ascending - each slice
            # feeds one pair-k-step burst).
            w1_first = w1p.tile([P, 2, KO, P], bf16, name="w1_t")
            nc.scalar.dma_start(w1_first[:], w1_r[0])
            xe_sb = xep.tile([P, KO, c_act], bf16)
            ko_step = 2 if KO % 2 == 0 else 1
            for kp in range(0, KO, ko_step):
                nc.sync.dma_start(
                    xe_sb[:, kp : kp + ko_step, :], xeT_r[:, kp : kp + ko_step, :]
                )

            # HAM warm-up: dependency-free matmuls fill the PE activity
            # window during the xe/w1 stream-in, so real matmuls start at
            # 2.4GHz instead of the cold 1.2GHz
            warm_ps = warmp.tile([P, 512], f32)
            for _ in range(N_WARM):
                nc.tensor.matmul(
                    warm_ps[:], warm_w[:], warm_sb[:], start=True, stop=True
                )

            y_sb = yp.tile([P, CS, D], f32)
            y_bf = yp.tile([P, CS, D], bf16)

            # chunk index -> (c offset, width)
            offs = []
            c0 = 0
            for cw in chunks:
                offs.append((c0, cw))
                c0 += cw
            idxs = list(range(len(chunks)))

            def mm1_sweep(g, use_first):
                """MM1 over this group's f-tiles, processed in pairs with a
                k-major inner loop (keeps the PE paced with the xe stream
                during startup; steady-state cycle count is unchanged)."""
                for pr in range(NPR):
                    fcs = [fc for fc in (2 * pr, 2 * pr + 1) if fc < FPG]
                    npr_g = g * NPR + pr
                    if use_first and pr == 0:
                        w1_t = w1_first
                    else:
                        w1_t = w1p.tile([P, 2, KO, P], bf16, name="w1_t")
                        # pairs 1-2 ride the sync ring behind the xe slices
                        # (they land just before they're needed, without
                        # stealing early bandwidth); the rest stream on the
                        # scalar ring paced by pool recycling
                        eng = nc.sync if npr_g in (1, 2) else nc.scalar
                        eng.dma_start(w1_t[:], w1_r[npr_g])
                    ph = {
                        (fc, i): psp.tile(
                            [P, chunks[i]], f32, name=f"p{i}", tag=f"p{i}"
                        )
                        for fc in fcs
                        for i in idxs
                    }
                    for k in range(KO):
                        for fc in fcs:
                            for i in idxs:
                                c0, cw = offs[i]
                                nc.tensor.matmul(
                                    ph[(fc, i)][:],
                                    w1_t[:, fc % 2, k, :],
                                    xe_sb[:, k, c0 : c0 + cw],
                                    start=(k == 0),
                                    stop=(k == KO - 1),
                                )
                    for fc in fcs:
                        for i in idxs:
                            c0, cw = offs[i]
                            nc.scalar.activation(
                                hT[:, fc, c0 : c0 + cw],
                                ph[(fc, i)][:],
                                Relu,
                                bias=zero[:],
                            )

            def mm2_cs(cs, w2_t, g, dh_list, add_strips=1):
                """MM2 accumulation over this group's f-chunks for one
                c-subtile, then fold into y_sb (bf16 y_bf on last group).
                add_strips > 1 splits the fold column-wise so the very last
                strip's add+store chain is short."""
                py = {
                    dh: psp.tile([P, 512], f32, name=f"py{dh}", tag=f"p{dh}")
                    for dh in dh_list
                }
                for fs in range(FPG):
                    for dh in dh_list:
                        nc.tensor.matmul(
                            py[dh][:],
                            hT[:, fs, cs * P : (cs + 1) * P],
                            w2_t[:, fs, dh * 512 : (dh + 1) * 512],
                            start=(fs == 0),
                            stop=(fs == FPG - 1),
                        )
                for dh in dh_list:
                    w = 512 // add_strips
                    for s in range(add_strips):
                        sl = slice(dh * 512 + s * w, dh * 512 + (s + 1) * w)
                        psl = slice(s * w, (s + 1) * w)
                        if g == 0:
                            nc.vector.tensor_copy(y_sb[:, cs, sl], py[dh][:, psl])
                        elif g < G - 1:
                            nc.vector.tensor_add(
                                y_sb[:, cs, sl], y_sb[:, cs, sl], py[dh][:, psl]
                            )
                        else:
                            nc.vector.tensor_add(
                                y_bf[:, cs, sl], y_sb[:, cs, sl], py[dh][:, psl]
                            )

            for g in range(G):
                # ---- MM1: hT[group] = relu(W1[:, group].T @ xeT) ----
                mm1_sweep(g, use_first=(g == 0))

                # W2 for this group: one batched transfer on the sync ring
                # (emitted after MM1 so the ring serves w1/xe first)
                w2_t = w2p.tile([P, FPG, D], bf16, name="w2_t")
                nc.sync.dma_start(w2_t[:], w2_r[:, g * FPG : (g + 1) * FPG, :])

                # ---- MM2: y[group contribution] = hT.T @ W2[group] ----
                for cs in range(CS):
                    last = g == G - 1
                    if last and cs == CS - 1:
                        # final c-subtile: run the D-halves as separate
                        # accumulation passes so the earlier halves'
                        # add+store hide under the next half's matmul
                        # stream; the very last half folds and stores in
                        # two column strips on both rings to shorten the
                        # exposed tail chain
                        for dh in range(ND):
                            final = dh == ND - 1
                            mm2_cs(cs, w2_t, g, [dh], add_strips=2 if final else 1)
                            if not final:
                                half = slice(dh * 512, (dh + 1) * 512)
                                eng = nc.sync if dh % 2 == 0 else nc.scalar
                                eng.dma_start(
                                    y[cs * P : (cs + 1) * P, half],
                                    y_bf[:, cs, half],
                                )
                            else:
                                for s in range(2):
                                    sl = slice(dh * 512 + s * 256, dh * 512 + (s + 1) * 256)
                                    eng = nc.sync if s == 0 else nc.scalar
                                    eng.dma_start(
                                        y[cs * P : (cs + 1) * P, sl],
                                        y_bf[:, cs, sl],
                                    )
                    else:
                        mm2_cs(cs, w2_t, g, list(range(ND)))
                        if last:
                            # batched store per c-subtile pair (alternating
                            # rings); a leftover even subtile before the
                            # final one stores alone
                            if cs % 2 == 1:
                                eng = nc.sync if cs % 4 == 1 else nc.scalar
                                # dst rearranged so element order matches the
                                # SBUF [p][two][d] tile layout
                                eng.dma_start(
                                    y[(cs - 1) * P : (cs + 1) * P, :].rearrange(
                                        "(two p) d -> p two d", p=P
                                    ),
                                    y_bf[:, cs - 1 : cs + 1, :],
                                )
                            elif cs == CS - 2:
                                nc.scalar.dma_start(
                                    y[cs * P : (cs + 1) * P, :],
                                    y_bf[:, cs, :],
                                )

    nc.compile()
    names = dict(w1=w1.name, w2=w2.name, xeT=xeT.name, y=y.name)
    return nc, names


def _get_program(c_act, c_cmp):
    key = (c_act, c_cmp)
    if key not in _PROGRAMS:
        _PROGRAMS[key] = _build_program(c_act, c_cmp)
    return _PROGRAMS[key]


# test.py can set RUN_KWARGS (e.g. dict(trace=True)) and read LAST_RESULTS
RUN_KWARGS = {}
LAST_RESULTS = None


def kernel(x, route_mask, route_weight, W1, b1, W2, b2):
    from concourse.bass_utils import run_bass_kernel_spmd

    global LAST_RESULTS

    x = np.asarray(x, dtype=np.float32)
    route_mask = np.asarray(route_mask, dtype=bool)
    route_weight = np.asarray(route_weight, dtype=np.float32)
    W1 = np.asarray(W1, dtype=np.float32)
    W2 = np.asarray(W2, dtype=np.float32)
    b1 = np.asarray(b1, dtype=np.float32)
    b2 = np.asarray(b2, dtype=np.float32)
    if np.any(b1):
        raise NotImplementedError("nonzero b1 not supported")

    # --- routing: per-expert top-C tokens by route weight (ties -> lower idx) ---
    w_et = np.where(route_mask.T, route_weight.T, -np.inf)  # (E, T)
    order = np.argsort(-w_et, axis=1, kind="stable")[:, :C]  # (E, C) token ids
    vals = np.take_along_axis(w_et, order, axis=1)  # (E, C)
    valid = np.isfinite(vals)  # (E, C)
    gain = np.where(valid, vals, 0.0).astype(np.float32)  # (E, C)

    # active capacity: valid slots are a prefix (sorted by weight desc).
    # c_act (tile shapes) is 128-aligned; c_cmp (MM1 computed columns) is
    # 8-aligned. The CS=1 case keeps a whole c-subtile so MM2 shapes hold.
    n_e = valid.sum(axis=1)
    n_max = int(max(1, n_e.max()))
    c_act = min(C, (n_max + P - 1) // P * P)
    c_cmp = min(c_act, (n_max + 7) // 8 * 8)

    nc, names = _get_program(c_act, c_cmp)

    # --- dispatch: gather + fold gain, per expert ---
    in_maps = []
    for e in range(E):
        xe = x[order[e, :c_act]] * gain[e, :c_act][:, None]  # (Ca, D)
        xeT_np = np.ascontiguousarray(xe.T).astype(BF16)  # (D, Ca)
        w1b = np.ascontiguousarray(
            W1[e].reshape(KO, P, NF, P).transpose(2, 1, 0, 3)
        ).astype(BF16)  # (NF, P, KO, P)
        in_maps.append(
            {
                names["w1"]: w1b,
                names["xeT"]: xeT_np,
                names["w2"]: W2[e].astype(BF16),
            }
        )

    res = run_bass_kernel_spmd(nc, in_maps, list(range(N_CORES)), **RUN_KWARGS)
    LAST_RESULTS = res

    # --- combine: scatter-add per-expert outputs ---
    y = np.zeros((T, D), np.float32)
    for e in range(E):
        ye = np.asarray(res.results[e][names["y"]]).astype(np.float32)  # (Ca, D)
        m = valid[e, :c_act]
        if np.any(b2):
            ye = ye + gain[e, :c_act][:, None] * b2[e][None, :]
        y[order[e, :c_act][m]] += ye[m]
    return y
